# revision 1
# baseline (speedup 1.0000x reference)
"""Trainium2 Bass kernel for a transformer decoder block (self-attn + cross-attn + FFN).

Sharding: zero-collective data parallelism over tokens. 8 cores; core c handles
batch b = c//4 and the 4 query blocks {4s + (c%4) : s in 0..3} of 128 tokens
each. Each core redundantly computes full-sequence K/V projections (cheaper
than on-chip collectives at this size) and everything else only for its own
tokens. The causal-attention structure is uniform across cores (one SPMD
program); per-core causal masks arrive as input data and are added to scores
with identity matmuls.

Layouts: activations are kept "transposed" ([feature, token]) for matmuls so
weights are always the stationary operand; LayerNorm/softmax-denominator/
residual work happens in [token, feature] layout; PE transposes convert.
Scores are computed as S^T = K^T.T @ Q^T ([key, query]), so the AV matmul
lhsT=[V|ones] both contracts keys and produces the softmax denominator for
free in PSUM row 64.
"""
import sys
import numpy as np
import ml_dtypes

for _p in ('/opt/trn_rl_repo',):
    if _p not in sys.path:
        sys.path.append(_p)

import concourse.bass as bass
import concourse.tile as tile
from concourse import bacc, mybir
from concourse.masks import make_identity

P = 128
HD = 64
EPS = 1e-5
NEG = -1e9

f32 = mybir.dt.float32
f32r = mybir.dt.float32r
bf16 = mybir.dt.bfloat16
AF = mybir.ActivationFunctionType


class Cfg:
    def __init__(self, T=2048, D=1024, H=16, FF=4096):
        self.T, self.D, self.H, self.FF = T, D, H, FF
        self.OWN = T // 4          # tokens per core
        self.NQB = self.OWN // P   # own q-blocks (128 each)
        self.DC = D // P           # D chunks
        self.FC = FF // P          # FFN chunks
        self.PAIRS = H // 2
        self.KB = T // P           # key blocks (global)
        self.TH = T // 2           # tokens per half
        self.KBH = self.KB // 2    # key blocks per half
        assert self.OWN <= 512 and self.T % 256 == 0 and D % P == 0
        assert H % 2 == 0 and self.FC % 2 == 0
        # active mask positions: (s, k) that are not always-visible for
        # every core j: k >= NQB*s  (g_s = NQB*s + j >= NQB*s)
        self.mask_pos = [(s, k) for s in range(self.NQB)
                         for k in range(self.NQB * s, self.KB)]
        self.mask_idx = {sk: i for i, sk in enumerate(self.mask_pos)}


def build_masks(cfg, j):
    """Additive causal mask tiles for core j: [P, n_active*P] f32."""
    m = np.zeros((len(cfg.mask_pos), P, P), np.float32)
    for i, (s, k) in enumerate(cfg.mask_pos):
        g = cfg.NQB * s + j
        if k < g:
            continue                      # fully visible: additive zero
        elif k == g:
            pidx = np.arange(P)[:, None]  # key within block
            fidx = np.arange(P)[None, :]  # query within block
            m[i] = np.where(pidx <= fidx, 0.0, NEG)
        else:
            m[i] = NEG
    return np.ascontiguousarray(m.transpose(1, 0, 2).reshape(P, -1))


def build_nc(cfg, with_gb):
    T, D, H, FF = cfg.T, cfg.D, cfg.H, cfg.FF
    OWN, NQB, DC, FC = cfg.OWN, cfg.NQB, cfg.DC, cfg.FC
    PAIRS, KB, TH, KBH = cfg.PAIRS, cfg.KB, cfg.TH, cfg.KBH
    NACT = len(cfg.mask_pos)
    scale = float(D) ** -0.5
    HW = H * HD                      # width of all-heads V
    VCH = (HW + 511) // 512          # 512-wide chunks of it

    nc = bacc.Bacc("TRN2", target_bir_lowering=False, debug=False)
    dp = nc.declare_dram_parameter
    x_dec = dp("x_dec", [T, D], f32, isOutput=False)
    x_enc = dp("x_enc", [T, D], f32, isOutput=False)
    x_own = dp("x_own", [OWN, D], f32, isOutput=False)
    wq_sa = dp("wq_sa", [PAIRS, D, P], f32r, isOutput=False)
    wk_sa = dp("wk_sa", [PAIRS, D, P], f32r, isOutput=False)
    wv_sa = dp("wv_sa", [D, HW], f32r, isOutput=False)
    wo_sa = dp("wo_sa", [D, D], f32r, isOutput=False)
    bo_sa = dp("bo_sa", [D], f32, isOutput=False)
    wq_ca = dp("wq_ca", [PAIRS, D, P], f32r, isOutput=False)
    wk_ca = dp("wk_ca", [PAIRS, D, P], f32r, isOutput=False)
    wv_ca = dp("wv_ca", [D, HW], f32r, isOutput=False)
    wo_ca = dp("wo_ca", [D, D], f32r, isOutput=False)
    bo_ca = dp("bo_ca", [D], f32, isOutput=False)
    w1 = dp("w1", [D, FF], f32r, isOutput=False)
    b1 = dp("b1", [FF], f32, isOutput=False)
    w2 = dp("w2", [FF, D], f32r, isOutput=False)
    b2 = dp("b2", [D], f32, isOutput=False)
    masks = dp("masks", [P, NACT * P], bf16, isOutput=False)
    gbs = {}
    if with_gb:
        for n in ("g1", "be1", "g2", "be2", "g3", "be3"):
            gbs[n] = dp(n, [D], f32, isOutput=False)
    out = dp("out", [OWN, D], f32, isOutput=True)

    vsc_sa = nc.dram_tensor("vsc_sa", [T, H * 65], f32r)
    vsc_ca = nc.dram_tensor("vsc_ca", [T, H * 65], f32r)

    r = lambda ap: ap.bitcast(f32r)

    from contextlib import ExitStack
    with tile.TileContext(nc) as tc:
        with ExitStack() as _ctx:
            _ctx.enter_context(nc.allow_low_precision(
                reason="float32r matmul inputs (fp32r rounds ~fp32)"))
            _pool = lambda nm, bufs, **kw: _ctx.enter_context(
                tc.tile_pool(name=nm, bufs=bufs, **kw))
            constp = _pool("constp", 1)
            xTp = _pool("xTp", 1)
            lnqp = _pool("lnqp", 1)
            ktpp = _pool("ktpp", 2)
            qTp = _pool("qTp", 8)
            vstp = _pool("vstp", 2)
            avpp = _pool("avpp", 8)
            residp = _pool("residp", 4)
            pbp = _pool("pbp", 2)
            ldp = _pool("ldp", 2)
            evp = _pool("evp", 3)
            wpairp = _pool("wpairp", 3)
            wbigp = _pool("wbigp", 2)
            smallp = _pool("smallp", 8)
            bp = _pool("bp", 1)
            scps = _pool("scps", 2, space="PSUM")
            avps = _pool("avps", 2, space="PSUM")
            linps = _pool("linps", 2, space="PSUM")
            # ---------------- constants ----------------
            ident = constp.tile([P, P], f32, tag="ident")
            make_identity(nc, ident[:])
            identb = constp.tile([P, P], bf16, tag="identb")
            make_identity(nc, identb[:])
            ones65f = constp.tile([65, HD], f32, tag="ones65f")
            nc.any.memset(ones65f[:], 1.0)
            ones65 = constp.tile([65, HD], f32r, tag="ones65")
            nc.vector.tensor_copy(ones65[:], ones65f[:])
            ones8 = constp.tile([P, 8], f32, tag="ones8")
            nc.any.memset(ones8[:], 1.0)
            epst = constp.tile([P, 1], f32, tag="epst")
            nc.any.memset(epst[:], EPS)
            maskt = constp.tile([P, NACT * P], bf16, tag="maskt")
            nc.sync.dma_start(maskt[:], masks[:])

            def load_bias_T(dram, n):
                """[n*P] DRAM vector -> [P, n] tile (chunk c in column c)."""
                t = bp.tile([P, n], f32, tag=dram.tensor.name + "_t")
                nc.sync.dma_start(t[:], dram[:].rearrange("(d p) -> p d", p=P))
                return t

            gb_tiles = {}
            if with_gb:
                for gk, bk, key in (("g1", "be1", 1), ("g2", "be2", 2),
                                    ("g3", "be3", 3)):
                    gb_tiles[key] = (load_bias_T(gbs[gk][:], DC),
                                    load_bias_T(gbs[bk][:], DC))
            bo_sa_t = load_bias_T(bo_sa[:], DC)
            bo_ca_t = load_bias_T(bo_ca[:], DC)
            b1_t = load_bias_T(b1[:], FC)
            b2_t = load_bias_T(b2[:], DC)

            # ---------------- helpers ----------------
            def emit_ln(xt, gbkey, inplace=True):
                """LayerNorm over D (gamma/beta folded at transpose-evict)."""
                nch = (D + 511) // 512
                st6 = smallp.tile([P, nch * 6], f32, tag="st6")
                for i in range(nch):
                    c0, c1 = i * 512, min(D, (i + 1) * 512)
                    nc.vector.bn_stats(st6[:, i * 6:(i + 1) * 6], xt[:, c0:c1])
                mv = smallp.tile([P, 2], f32, tag="mv")
                nc.vector.bn_aggr(mv[:], st6[:].rearrange("p (a b) -> p a b",
                                                          b=6))
                std = smallp.tile([P, 1], f32, tag="std")
                nc.scalar.activation(std[:], mv[:, 1:2], AF.Sqrt, bias=epst[:])
                rstd = smallp.tile([P, 1], f32, tag="rstd")
                nc.vector.reciprocal(rstd[:], std[:])
                mrs = smallp.tile([P, 1], f32, tag="mrs")
                nc.vector.tensor_mul(mrs[:], mv[:, 0:1], rstd[:])
                nmrs = smallp.tile([P, 1], f32, tag="nmrs")
                nc.vector.tensor_scalar_mul(nmrs[:], mrs[:], -1.0)
                if inplace:
                    lnt = xt
                else:
                    lnt = ldp.tile([P, D], f32, tag="ld")
                nc.scalar.activation(lnt[:], xt[:], AF.Identity,
                                     bias=nmrs[:], scale=rstd[:])
                return lnt

            def emit_transposes(src, dst_view, gbkey):
                """Transpose [P, D] src into dst_view [P, DC, P] (d-major),
                packing 4 transposes per PSUM tile."""
                for g0 in range(0, DC, 4):
                    gn = min(4, DC - g0)
                    ps = linps.tile([P, 512], f32, tag="lin")
                    for i in range(gn):
                        d = g0 + i
                        nc.tensor.transpose(ps[:, i * P:(i + 1) * P],
                                            src[:, d * P:(d + 1) * P],
                                            ident[:])
                    if with_gb and gbkey is not None:
                        gt, bt = gb_tiles[gbkey]
                        for i in range(gn):
                            d = g0 + i
                            nc.scalar.activation(
                                dst_view[:, d, :], ps[:, i * P:(i + 1) * P],
                                AF.Identity, bias=bt[:, d:d + 1],
                                scale=gt[:, d:d + 1])
                    else:
                        nc.vector.tensor_copy(dst_view[:, g0:g0 + gn, :],
                                              ps[:, :gn * P])

            def ln_transpose_stream(src_dram, row0, nrows, xT, gbkey):
                """Stream [nrows, D] from DRAM (rows row0..), LN if gbkey,
                transpose into mega-tile xT ([P, DC*TH], token col = local)."""
                xTv = xT[:].rearrange("p (d t) -> p d t", t=TH)
                for tb in range(nrows // P):
                    xt = ldp.tile([P, D], f32, tag="ld")
                    nc.sync.dma_start(
                        xt[:], src_dram[row0 + tb * P:row0 + (tb + 1) * P, :])
                    lnt = emit_ln(xt, gbkey) if gbkey is not None else xt
                    emit_transposes(lnt, xTv[:, :, tb * P:(tb + 1) * P], gbkey)

            def emit_pair_proj(w_dram, pair, rhs_slices, rhs_w, dst, dst_c0):
                """dst[:, c] = w_pair.T @ rhs ([128=2 heads] rows), contracting
                D in 128-chunks. rhs_slices(d, c0, cw) -> AP."""
                wt = wpairp.tile([P, DC * P], f32r, tag="wpair")
                nc.sync.dma_start(
                    wt[:].rearrange("p (d c) -> p d c", c=P),
                    w_dram[pair].rearrange("(d p) c -> p d c", p=P))
                for c0 in range(0, rhs_w, 512):
                    cw = min(512, rhs_w - c0)
                    ps = linps.tile([P, 512], f32, tag="lin")
                    for d in range(DC):
                        nc.tensor.matmul(ps[:, :cw], r(wt[:, d * P:(d + 1) * P]),
                                         r(rhs_slices(d, c0, cw)),
                                         start=(d == 0), stop=(d == DC - 1))
                    nc.vector.tensor_copy(dst[:, dst_c0 + c0:dst_c0 + c0 + cw],
                                          ps[:, :cw])

            def emit_v_to_scratch(wv_dram, xT, half, vsc):
                """V in natural layout for all heads -> DRAM scratch; each
                head gets 65 columns with col 64 = 1.0 (softmax denominator
                rides the AV matmul for free)."""
                for nch in range(VCH):
                    c0, cw = nch * 512, min(512, HW - nch * 512)
                    nh = cw // HD
                    wvh = wbigp.tile([P, DC * 512], f32r, tag="wbig",
                                     name=f"wvh{half}{nch}")
                    nc.sync.dma_start(
                        wvh[:, :DC * cw].rearrange("p (d c) -> p d c", c=cw),
                        wv_dram[:, c0:c0 + cw].rearrange("(d p) c -> p d c",
                                                         p=P))
                    for tb in range(TH // P):
                        ps = linps.tile([P, 512], f32, tag="lin")
                        for d in range(DC):
                            nc.tensor.matmul(
                                ps[:, :cw],
                                r(xT[:, d * TH + tb * P:d * TH + (tb + 1) * P]),
                                r(wvh[:, d * cw:(d + 1) * cw]),
                                start=(d == 0), stop=(d == DC - 1))
                        ev = evp.tile([P, 8 * 65], f32r, tag="ev")
                        evv = ev[:, :nh * 65].rearrange("p (h c) -> p h c",
                                                        c=65)
                        nc.vector.tensor_copy(
                            evv[:, :, 64:65],
                            ones8[:, :nh].unsqueeze(2))
                        nc.scalar.activation(
                            evv[:, :, 0:HD],
                            ps[:, :cw].rearrange("p (h c) -> p h c", c=HD),
                            AF.Copy)
                        row0 = half * TH + tb * P
                        nc.sync.dma_start(
                            vsc[row0:row0 + P,
                                nch * 8 * 65:nch * 8 * 65 + nh * 65],
                            ev[:, :nh * 65])

            def emit_kt_pair(w_dram, pair, xT):
                """K^T for one head pair from transposed activations."""
                kt = ktpp.tile([P, TH], f32r, tag="ktp")
                emit_pair_proj(w_dram, pair,
                               lambda d, c0, cw: xT[:, d * TH + c0:
                                                    d * TH + c0 + cw],
                               TH, kt, 0)
                return kt

            def emit_attention(qT, kt, vsc, half, pair, avst, use_masks):
                """One half-T of attention, both heads of a pair.
                qT: [P, OWN] (rows 0:64 head A, 64:128 head B).
                kt: [P, TH]. avst: dict with rolling 'psum' + 'sbuf' [P,2*OWN]
                partial accumulator ([O^T;denom] per head in column halves)."""
                vtt = vstp.tile([P, KBH * 130], f32r, tag="vst")
                nc.sync.dma_start(
                    vtt[:].rearrange("p (kl c) -> p kl c", c=130),
                    vsc[half * TH:(half + 1) * TH,
                        pair * 130:(pair + 1) * 130]
                    .rearrange("(kl p) c -> p kl c", p=P))
                NG = (KBH + 1) // 2
                for hh in range(2):
                    hb = hh * HD
                    for g in range(NG):
                        kls = [kl for kl in (2 * g, 2 * g + 1) if kl < KBH]
                        sc = scps.tile([P, 2 * OWN], f32, tag="sc")
                        for i, kl in enumerate(kls):
                            kg = half * KBH + kl
                            mss = [s for s in range(NQB)
                                   if use_masks and (s, kg) in cfg.mask_idx]
                            nc.tensor.matmul(
                                sc[:, i * OWN:(i + 1) * OWN],
                                r(kt[hb:hb + HD, kl * P:(kl + 1) * P]),
                                r(qT[hb:hb + HD, :]),
                                start=True, stop=(not mss))
                            for n, s in enumerate(mss):
                                idx = cfg.mask_idx[(s, kg)]
                                nc.tensor.matmul(
                                    sc[:, i * OWN + s * P:
                                       i * OWN + (s + 1) * P],
                                    identb[:],
                                    maskt[:, idx * P:(idx + 1) * P],
                                    start=False, stop=(n == len(mss) - 1))
                        pb = pbp.tile([P, 2 * OWN], f32r, tag="pb")
                        nc.scalar.activation(pb[:, :len(kls) * OWN],
                                             sc[:, :len(kls) * OWN],
                                             AF.Exp, scale=scale)
                        for i, kl in enumerate(kls):
                            nc.tensor.matmul(
                                avst["psum"][:],
                                r(vtt[:, kl * 130 + hh * 65:
                                      kl * 130 + (hh + 1) * 65]),
                                r(pb[:, i * OWN:(i + 1) * OWN]),
                                start=(g == 0 and i == 0),
                                stop=(g == NG - 1 and i == len(kls) - 1))
                    dst = avst["sbuf"][0:65, hh * OWN:(hh + 1) * OWN]
                    if half == 0:
                        nc.vector.tensor_copy(dst, avst["psum"][:])
                    else:
                        nc.vector.tensor_add(dst, avst["psum"][:], dst)
                    if not (half == 1 and hh == 1):
                        avst["psum"] = avps.tile([65, OWN], f32, tag="av",
                                                 name=f"avps_{pair}_{half}_{hh}")

            def emit_normalize(avp_sb, hh):
                """O^T /= denominator row, in place in the sbuf partial."""
                cs = hh * OWN
                rec = evp.tile([P, 512], f32r, tag="ev")
                nc.vector.reciprocal(rec[64:65, :OWN],
                                     avp_sb[64:65, cs:cs + OWN])
                bc = avps.tile([64, OWN], f32, tag="av")
                nc.tensor.matmul(bc[:], r(ones65[64:65, :]),
                                 r(rec[64:65, :OWN]), start=True, stop=True)
                bcs = evp.tile([P, 512], f32, tag="ev")
                nc.scalar.activation(bcs[0:64, :OWN], bc[:], AF.Copy)
                nc.vector.tensor_mul(avp_sb[0:64, cs:cs + OWN],
                                     avp_sb[0:64, cs:cs + OWN],
                                     bcs[0:64, :OWN])

            def emit_oproj_residual(wo_dram, bo_t, avp_list, res_tiles):
                """res += transpose(Wo^T @ O^T + bo)   (residual in place)."""
                for m in range(DC):
                    wot = wbigp.tile([64, 2 * DC * P], f32r, tag="wbig",
                                     name=f"wot{m}")
                    nc.sync.dma_start(
                        wot[:].rearrange("p (a c) -> p a c", c=P),
                        wo_dram[:, m * P:(m + 1) * P]
                        .rearrange("(a p) c -> p a c", p=64))
                    ps = linps.tile([P, 512], f32, tag="lin")
                    for n in range(2 * DC):
                        pair, hh = n // 2, n % 2
                        nc.tensor.matmul(
                            ps[:, :OWN], r(wot[:, n * P:(n + 1) * P]),
                            r(avp_list[pair][0:64, hh * OWN:(hh + 1) * OWN]),
                            start=(n == 0), stop=(n == 2 * DC - 1))
                    ev = evp.tile([P, 512], f32, tag="ev")
                    nc.scalar.activation(ev[:, :OWN], ps[:, :OWN], AF.Identity,
                                         bias=bo_t[:, m:m + 1])
                    ps2 = linps.tile([P, 512], f32, tag="lin")
                    for s in range(NQB):
                        nc.tensor.transpose(ps2[:, s * P:(s + 1) * P],
                                            ev[:, s * P:(s + 1) * P], ident[:])
                    for s in range(NQB):
                        nc.vector.tensor_add(
                            res_tiles[s][:, m * P:(m + 1) * P],
                            ps2[:, s * P:(s + 1) * P],
                            res_tiles[s][:, m * P:(m + 1) * P])

            def emit_lnq(res_tiles_or_dram, gbkey, from_dram):
                """LN own tokens + transpose -> [P, DC*OWN] mega-tile."""
                lnq = lnqp.tile([P, DC * OWN], f32r, tag="lnq")
                lnqv = lnq[:].rearrange("p (d t) -> p d t", t=OWN)
                for s in range(NQB):
                    if from_dram:
                        xt = ldp.tile([P, D], f32, tag="ld")
                        nc.sync.dma_start(
                            xt[:], res_tiles_or_dram[s * P:(s + 1) * P, :])
                    else:
                        xt = res_tiles_or_dram[s]
                    lnt = emit_ln(xt[:], gbkey, inplace=from_dram)
                    emit_transposes(lnt, lnqv[:, :, s * P:(s + 1) * P], gbkey)
                return lnq

            # ================= pipeline =================
            res = []
            for s in range(NQB):
                t = residp.tile([P, D], f32, tag="resid")
                nc.sync.dma_start(t[:], x_own[s * P:(s + 1) * P, :])
                res.append(t)

            # own-token LN1 -> Q_sa^T
            lnq1 = emit_lnq(x_own, 1, True)
            qsaT = []
            for pair in range(PAIRS):
                qt = qTp.tile([P, OWN], f32r, tag="qT")
                emit_pair_proj(
                    wq_sa, pair,
                    lambda d, c0, cw: lnq1[:, d * OWN + c0:d * OWN + c0 + cw],
                    OWN, qt, 0)
                qsaT.append(qt)

            # SA attention in two half-T passes
            av_sa = [avpp.tile([P, 2 * OWN], f32r, tag="avp", name=f"av_sa{_pp}")
                     for _pp in range(PAIRS)]
            avst_sa = {}
            for half in range(2):
                xT = xTp.tile([P, DC * TH], f32r, tag="xT")
                ln_transpose_stream(x_dec, half * TH, TH, xT, 1)
                emit_v_to_scratch(wv_sa, xT, half, vsc_sa)
                for pair in range(PAIRS):
                    kt = emit_kt_pair(wk_sa, pair, xT)
                    if half == 0:
                        avst_sa[pair] = {
                            "psum": avps.tile([65, OWN], f32, tag="av",
                                              name=f"avps_sa{pair}"),
                            "sbuf": av_sa[pair]}
                    emit_attention(qsaT[pair], kt, vsc_sa, half, pair,
                                   avst_sa[pair], True)
                    if half == 1:
                        emit_normalize(av_sa[pair], 0)
                        emit_normalize(av_sa[pair], 1)
            emit_oproj_residual(wo_sa, bo_sa_t, av_sa, res)     # res -> x2

            # LN2 -> Q_ca^T
            lnq2 = emit_lnq(res, 2, False)
            qcaT = []
            for pair in range(PAIRS):
                qt = qTp.tile([P, OWN], f32r, tag="qT")
                emit_pair_proj(
                    wq_ca, pair,
                    lambda d, c0, cw: lnq2[:, d * OWN + c0:d * OWN + c0 + cw],
                    OWN, qt, 0)
                qcaT.append(qt)

            # CA attention (raw encoder K/V, no masks)
            av_ca = [avpp.tile([P, 2 * OWN], f32r, tag="avp", name=f"av_ca{_pp}")
                     for _pp in range(PAIRS)]
            avst_ca = {}
            for half in range(2):
                xT = xTp.tile([P, DC * TH], f32r, tag="xT")
                ln_transpose_stream(x_enc, half * TH, TH, xT, None)
                emit_v_to_scratch(wv_ca, xT, half, vsc_ca)
                for pair in range(PAIRS):
                    kt = emit_kt_pair(wk_ca, pair, xT)
                    if half == 0:
                        avst_ca[pair] = {
                            "psum": avps.tile([65, OWN], f32, tag="av",
                                              name=f"avps_ca{pair}"),
                            "sbuf": av_ca[pair]}
                    emit_attention(qcaT[pair], kt, vsc_ca, half, pair,
                                   avst_ca[pair], False)
                    if half == 1:
                        emit_normalize(av_ca[pair], 0)
                        emit_normalize(av_ca[pair], 1)
            emit_oproj_residual(wo_ca, bo_ca_t, av_ca, res)     # res -> x3

            # LN3 -> FFN
            lnq3 = emit_lnq(res, 3, False)
            y2T = [qTp.tile([P, OWN], f32, tag="qT", name=f"y2T{_m}") for _m in range(DC)]
            FG = FC // 2
            for fg in range(2):
                rT = xTp.tile([P, DC * TH], f32r, tag="xT")
                for fi in range(FG):
                    f = fg * FG + fi
                    w1t = wpairp.tile([P, DC * P], f32r, tag="wpair",
                                      name=f"w1t{f}")
                    nc.sync.dma_start(
                        w1t[:].rearrange("p (d c) -> p d c", c=P),
                        w1[:, f * P:(f + 1) * P]
                        .rearrange("(d p) c -> p d c", p=P))
                    ps = linps.tile([P, 512], f32, tag="lin")
                    for d in range(DC):
                        nc.tensor.matmul(
                            ps[:, :OWN], r(w1t[:, d * P:(d + 1) * P]),
                            r(lnq3[:, d * OWN:(d + 1) * OWN]),
                            start=(d == 0), stop=(d == DC - 1))
                    nc.scalar.activation(rT[:, fi * OWN:(fi + 1) * OWN],
                                         ps[:, :OWN], AF.Relu,
                                         bias=b1_t[:, f:f + 1])
                for m in range(DC):
                    w2t = wbigp.tile([P, FG * P], f32r, tag="wbig",
                                     name=f"w2t{fg}{m}")
                    nc.sync.dma_start(
                        w2t[:].rearrange("p (a c) -> p a c", c=P),
                        w2[fg * FG * P:(fg + 1) * FG * P, m * P:(m + 1) * P]
                        .rearrange("(a p) c -> p a c", p=P))
                    ps = linps.tile([P, 512], f32, tag="lin")
                    for fi in range(FG):
                        nc.tensor.matmul(
                            ps[:, :OWN], r(w2t[:, fi * P:(fi + 1) * P]),
                            r(rT[:, fi * OWN:(fi + 1) * OWN]),
                            start=(fi == 0), stop=(fi == FG - 1))
                    if fg == 0:
                        nc.scalar.activation(y2T[m][:], ps[:, :OWN],
                                             AF.Identity,
                                             bias=b2_t[:, m:m + 1])
                    else:
                        nc.vector.tensor_add(y2T[m][:], ps[:, :OWN],
                                             y2T[m][:])

            for m in range(DC):
                ps2 = linps.tile([P, 512], f32, tag="lin")
                for s in range(NQB):
                    nc.tensor.transpose(ps2[:, s * P:(s + 1) * P],
                                        y2T[m][:, s * P:(s + 1) * P], ident[:])
                for s in range(NQB):
                    nc.vector.tensor_add(res[s][:, m * P:(m + 1) * P],
                                         ps2[:, s * P:(s + 1) * P],
                                         res[s][:, m * P:(m + 1) * P])
            for s in range(NQB):
                nc.sync.dma_start(out[s * P:(s + 1) * P, :], res[s][:])

    nc.compile()
    return nc


def own_token_rows(cfg, j):
    return np.concatenate(
        [np.arange(P * (cfg.NQB * s + j), P * (cfg.NQB * s + j) + P)
         for s in range(cfg.NQB)])


def prep_core_inputs(cfg, inputs, core):
    """Host-side slicing/packing for one core."""
    D, H = cfg.D, cfg.H
    b, j = core // 4, core % 4
    a = lambda x: np.asarray(x)
    f32c = lambda x: np.ascontiguousarray(a(x), dtype=np.float32)
    pack_pairs = lambda w: np.ascontiguousarray(np.stack(
        [np.concatenate([a(w)[2 * p], a(w)[2 * p + 1]], axis=1)
         for p in range(cfg.PAIRS)]), dtype=np.float32)
    vall = lambda w: np.ascontiguousarray(
        a(w).transpose(1, 0, 2).reshape(D, H * HD), dtype=np.float32)

    rows = own_token_rows(cfg, j)
    return {
        "x_dec": f32c(a(inputs["decoder_x"])[b]),
        "x_enc": f32c(a(inputs["encoder_x"])[b]),
        "x_own": f32c(a(inputs["decoder_x"])[b][rows]),
        "wq_sa": pack_pairs(inputs["Wq_sa"]),
        "wk_sa": pack_pairs(inputs["Wk_sa"]),
        "wv_sa": vall(inputs["Wv_sa"]),
        "wo_sa": f32c(inputs["Wo_sa"]),
        "bo_sa": f32c(inputs["bo_sa"]),
        "wq_ca": pack_pairs(inputs["Wq_ca"]),
        "wk_ca": pack_pairs(inputs["Wk_ca"]),
        "wv_ca": vall(inputs["Wv_ca"]),
        "wo_ca": f32c(inputs["Wo_ca"]),
        "bo_ca": f32c(inputs["bo_ca"]),
        "w1": f32c(inputs["W1"]),
        "b1": f32c(inputs["b1"]),
        "w2": f32c(inputs["W2"]),
        "b2": f32c(inputs["b2"]),
        "masks": build_masks(cfg, j).astype(ml_dtypes.bfloat16),
    }, rows


def gb_trivial(inputs):
    return all(np.allclose(np.asarray(inputs[g]), 1.0)
               for g in ("g1", "g2", "g3")) and \
           all(np.allclose(np.asarray(inputs[b]), 0.0)
               for b in ("be1", "be2", "be3"))


def run(inputs, trace=False, **rk):
    """Build + run on 8 cores; returns (full_output, BassKernelResults)."""
    from concourse.bass_utils import run_bass_kernel_spmd

    cfg = Cfg()
    with_gb = not gb_trivial(inputs)
    nc = build_nc(cfg, with_gb)

    in_maps, rows_all = [], []
    for core in range(8):
        im, rows = prep_core_inputs(cfg, inputs, core)
        if with_gb:
            for n in ("g1", "be1", "g2", "be2", "g3", "be3"):
                im[n] = np.ascontiguousarray(np.asarray(inputs[n]),
                                             dtype=np.float32)
        in_maps.append(im)
        rows_all.append(rows)

    res = run_bass_kernel_spmd(nc, in_maps, list(range(8)), trace=trace, **rk)
    full = np.zeros((2, cfg.T, cfg.D), np.float32)
    for core in range(8):
        full[core // 4][rows_all[core]] = res.results[core]["out"]
    return full, res


def kernel(**inputs) -> np.ndarray:
    return run(inputs)[0]



# revision 23
# speedup vs baseline: 1.0922x; 1.0922x over previous
"""Trainium2 Bass kernel for a transformer decoder block (self-attn + cross-attn + FFN).

Sharding: zero-collective data parallelism over tokens. 8 cores; core c handles
batch b = c//4 and the 4 query blocks {4s + (c%4) : s in 0..3} of 128 tokens
each. Each core redundantly computes full-sequence K/V projections and
everything else only for its own tokens. One SPMD program for all cores; the
causal structure is j-uniform (compute block (s, kl) iff kl <= 4s+3, the
superset over j) with per-core additive masks as input data.

v1 rewrite vs baseline:
- all matmul operands bf16 (1 cycle/row at any free-dim size; fp32r needs
  >=256 rows for that). PSUM accumulation and the residual stream stay fp32.
- full-T processing (no half split); V kept SBUF-resident in a per-pair
  [V_A(64) | 1 | 1 | V_B(64)] layout: the ones columns ride the AV matmuls to
  produce softmax denominators for free (head A's at PSUM partition 64, head
  B's at partition 63 so head B's output rows land on partitions 64:128).
  This kills the baseline's 34MB V DRAM round-trip.
- causal skipping: blocks with kl > 4s+3 are never computed (37.5% of SA
  score/AV/exp work gone); each visited kl gets one mask add at s = kl//4.
- packed O-projection: heads A/B stacked on partitions 0:64/64:128 gives a
  128-wide contraction (baseline contracted 64 partitions at a time).
- SA queries are sliced out of the LN1-transposed key stream (no separate
  own-token LN1 pass).
- weights host-packed into exact SBUF layout: every weight DMA is a fully
  contiguous [128, k*2KB] copy.
"""
import sys
import numpy as np
import ml_dtypes

for _p in ('/opt/trn_rl_repo',):
    if _p not in sys.path:
        sys.path.append(_p)

import concourse.bass as bass
import concourse.tile as tile
from concourse import bacc, mybir
from concourse.masks import make_identity

P = 128
HD = 64
EPS = 1e-5
NEG = -1e9

f32 = mybir.dt.float32
f32r = mybir.dt.float32r
bf16 = mybir.dt.bfloat16
AF = mybir.ActivationFunctionType


class Cfg:
    def __init__(self, T=2048, D=1024, H=16, FF=4096):
        self.T, self.D, self.H, self.FF = T, D, H, FF
        self.OWN = T // 4          # tokens per core
        self.NQB = self.OWN // P   # own q-blocks (128 each)
        self.DC = D // P           # D chunks
        self.FC = FF // P          # FFN chunks
        self.PAIRS = H // 2
        self.KB = T // P           # key blocks (global)
        self.VCH = (H * HD + 511) // 512


def build_nc(cfg, with_gb):
    T, D, H, FF = cfg.T, cfg.D, cfg.H, cfg.FF
    OWN, NQB, DC, FC = cfg.OWN, cfg.NQB, cfg.DC, cfg.FC
    PAIRS, KB, VCH = cfg.PAIRS, cfg.KB, cfg.VCH
    scale = float(D) ** -0.5

    nc = bacc.Bacc("TRN2", target_bir_lowering=False, debug=False)
    dp = nc.declare_dram_parameter
    x_dec = dp("x_dec", [T, D], f32, isOutput=False)
    x_enc = dp("x_enc", [T, D], f32, isOutput=False)
    x_own = dp("x_own", [OWN, D], f32, isOutput=False)
    wq_sa = dp("wq_sa", [P, PAIRS * DC * P], bf16, isOutput=False)
    wk_sa = dp("wk_sa", [P, PAIRS * DC * P], bf16, isOutput=False)
    wv_sa = dp("wv_sa", [P, VCH * DC * 512], bf16, isOutput=False)
    wo_sa = dp("wo_sa", [P, DC * PAIRS * P], bf16, isOutput=False)
    bo_sa = dp("bo_sa", [D], f32, isOutput=False)
    wq_ca = dp("wq_ca", [P, PAIRS * DC * P], bf16, isOutput=False)
    wk_ca = dp("wk_ca", [P, PAIRS * DC * P], bf16, isOutput=False)
    wv_ca = dp("wv_ca", [P, VCH * DC * 512], bf16, isOutput=False)
    wo_ca = dp("wo_ca", [P, DC * PAIRS * P], bf16, isOutput=False)
    bo_ca = dp("bo_ca", [D], f32, isOutput=False)
    w1 = dp("w1", [P, FC * DC * P], bf16, isOutput=False)
    b1 = dp("b1", [FF], f32, isOutput=False)
    w2 = dp("w2", [P, DC * FC * P], bf16, isOutput=False)
    b2 = dp("b2", [D], f32, isOutput=False)
    masks = dp("masks", [P, KB * P], bf16, isOutput=False)
    gbs = {}
    if with_gb:
        for n in ("g1", "be1", "g2", "be2", "g3", "be3"):
            gbs[n] = dp(n, [D], f32, isOutput=False)
    out = dp("out", [OWN, D], f32, isOutput=True)

    # j-uniform causal-skip table: visit (s, kl) iff kl <= 4s+3
    def s0_of(kl):
        return max(0, -(-(kl - (NQB - 1)) // NQB))

    from contextlib import ExitStack
    with tile.TileContext(nc) as tc:
        with ExitStack() as _ctx:
            _ctx.enter_context(nc.allow_low_precision(
                reason="bf16 matmul operands, fp32 accumulation"))
            _pool = lambda nm, bufs, **kw: _ctx.enter_context(
                tc.tile_pool(name=nm, bufs=bufs, **kw))
            constp = _pool("constp", 1)
            bp = _pool("bp", 1)
            ldbp = _pool("ldbp", 3)
            xTp = _pool("xTp", 1)
            vsbp = _pool("vsbp", 1)
            ktp = _pool("ktp", 2)
            qtp = _pool("qtp", 2)
            avkp = _pool("avkp", 8)
            lnqp = _pool("lnqp", 1)
            rtp = _pool("rtp", 1)
            residp = _pool("residp", 4)
            wp = _pool("wp", 2)
            wvp = _pool("wvp", 1)
            wop = _pool("wop", 2)
            w2p = _pool("w2p", 2)
            smallp = _pool("smallp", 8)
            pbp = _pool("pbp", 2)
            evp = _pool("evp", 2)
            normp = _pool("normp", 1)
            tmpp = _pool("tmpp", 1)
            linps = _pool("linps", 2, space="PSUM")
            scps = _pool("scps", 2, space="PSUM")
            avps = _pool("avps", 2, space="PSUM")

            # ---------------- constants ----------------
            identb = constp.tile([P, P], bf16, tag="identb")
            make_identity(nc, identb[:])
            onesf = constp.tile([P, HD], f32, tag="onesf")
            nc.any.memset(onesf[:], 1.0)
            onesr = constp.tile([P, HD], f32r, tag="onesr")
            nc.vector.tensor_copy(onesr[:], onesf[:])
            epst = constp.tile([P, 1], f32, tag="epst")
            nc.any.memset(epst[:], EPS)
            maskt = constp.tile([P, KB * P], bf16, tag="maskt")
            nc.sync.dma_start(maskt[:], masks[:])
            ro = lambda ap: ap.bitcast(f32r)

            def load_bias_T(dram, n):
                t = bp.tile([P, n], f32, tag=dram.tensor.name + "_t")
                nc.sync.dma_start(t[:], dram[:].rearrange("(d p) -> p d", p=P))
                return t

            gb_tiles = {}
            if with_gb:
                for gk, bk, key in (("g1", "be1", 1), ("g2", "be2", 2),
                                    ("g3", "be3", 3)):
                    gb_tiles[key] = (load_bias_T(gbs[gk][:], DC),
                                     load_bias_T(gbs[bk][:], DC))
            bo_sa_t = load_bias_T(bo_sa[:], DC)
            bo_ca_t = load_bias_T(bo_ca[:], DC)
            b1_t = load_bias_T(b1[:], FC)
            b2_t = load_bias_T(b2[:], DC)

            xT_cur = [None]

            # ---------------- helpers ----------------
            def emit_ln(xt):
                """LayerNorm stats+apply over D -> new bf16 tile [P, D].
                gamma/beta (if nontrivial) fold in at transpose-evict."""
                nch = (D + 511) // 512
                st6 = smallp.tile([P, nch * 6], f32, tag="st6")
                for i in range(nch):
                    c0, c1 = i * 512, min(D, (i + 1) * 512)
                    nc.vector.bn_stats(st6[:, i * 6:(i + 1) * 6], xt[:, c0:c1])
                mv = smallp.tile([P, 2], f32, tag="mv")
                nc.vector.bn_aggr(mv[:], st6[:].rearrange("p (a b) -> p a b",
                                                          b=6))
                std = smallp.tile([P, 1], f32, tag="std")
                nc.scalar.activation(std[:], mv[:, 1:2], AF.Sqrt, bias=epst[:])
                rstd = smallp.tile([P, 1], f32, tag="rstd")
                nc.vector.reciprocal(rstd[:], std[:])
                mrs = smallp.tile([P, 1], f32, tag="mrs")
                nc.vector.tensor_mul(mrs[:], mv[:, 0:1], rstd[:])
                nmrs = smallp.tile([P, 1], f32, tag="nmrs")
                nc.vector.tensor_scalar_mul(nmrs[:], mrs[:], -1.0)
                lnt = ldbp.tile([P, D], bf16, tag="ldb")
                nc.scalar.activation(lnt[:], xt[:], AF.Identity,
                                     bias=nmrs[:], scale=rstd[:])
                return lnt

            def emit_transposes(src, dst_view, gbkey):
                """Transpose bf16 [P, D] src into dst_view [P, DC, P]
                (d-major); all DC transposes through one bf16-bitcast PSUM
                tile, single evict."""
                ps = linps.tile([P, 512], f32, tag="lin")
                psb = ps[:].bitcast(bf16)          # [P, 1024] bf16 view
                for d in range(DC):
                    nc.tensor.transpose(psb[:, d * P:(d + 1) * P],
                                        src[:, d * P:(d + 1) * P],
                                        identb[:])
                if with_gb and gbkey is not None:
                    gt, bt = gb_tiles[gbkey]
                    for d in range(DC):
                        nc.scalar.activation(
                            dst_view[:, d, :], psb[:, d * P:(d + 1) * P],
                            AF.Identity, bias=bt[:, d:d + 1],
                            scale=gt[:, d:d + 1])
                else:
                    nc.vector.tensor_copy(dst_view[:, :, :],
                                          psb[:].rearrange(
                                              "p (d c) -> p d c", c=P))

            def stream_to_xT(src_dram, gbkey, wv_dram, which):
                """Stream [T, D] from DRAM (cast to bf16 in the DMA),
                optional LN, transpose into a [P, DC*T] bf16 mega-tile.
                The V projection for each token block is interleaved right
                after its transposes so PE chews V work while the LN chain
                (DVE/ACT-paced) produces the next block."""
                xT = xTp.tile([P, DC * T], bf16, tag="xT")
                xTv = xT[:].rearrange("p (d t) -> p d t", t=T)
                vsb = vsbp.tile([P, KB * PAIRS * 130], bf16, tag="vsb")
                v4 = vsb[:].rearrange("p (k r h c) -> p k r h c", r=PAIRS,
                                      h=2, c=65)
                nc.any.memset(v4[:, :, :, :, 64:65], 1.0)
                wvh = wvp.tile([P, VCH * DC * 512], bf16, tag="wvh",
                               name=f"wvh_{which}")
                for nch in range(VCH):
                    nc.sync.dma_start(
                        wvh[:, nch * DC * 512:(nch + 1) * DC * 512],
                        wv_dram[:, nch * DC * 512:(nch + 1) * DC * 512])
                for tb in range(T // P):
                    xt = ldbp.tile([P, D], bf16, tag="ldb")
                    nc.gpsimd.dma_start(
                        xt[:], src_dram[tb * P:(tb + 1) * P, :])
                    lnt = emit_ln(xt) if gbkey is not None else xt
                    emit_transposes(lnt, xTv[:, :, tb * P:(tb + 1) * P],
                                    gbkey)
                    for nch in range(VCH):
                        ps = linps.tile([P, 512], f32, tag="lin")
                        for d in range(DC):
                            nc.tensor.matmul(
                                ps[:],
                                xT[:, d * T + tb * P:d * T + (tb + 1) * P],
                                wvh[:, (nch * DC + d) * 512:
                                    (nch * DC + d + 1) * 512],
                                start=(d == 0), stop=(d == DC - 1))
                        srcv = ps[:].rearrange("p (r two c) -> p r two c",
                                               two=2, c=HD)
                        dstv = v4[:, tb, 4 * nch:4 * nch + 4, :, 0:HD]
                        nc.scalar.activation(dstv[:, :, :, :], srcv[:],
                                             AF.Copy)
                return xT, vsb

            def kq_steps(pair, which, wk_dram, wq_dram, rhs_of):
                """Emittable step closures that build kt/qt for `pair`;
                interleaved into the previous pair's kl loop so the PE fills
                ACT(exp)-pacing bubbles with projection work."""
                st = {}
                def s_dma():
                    wk = wp.tile([P, DC * P], bf16, tag="wx",
                                 name=f"wkt_{which}{pair}")
                    nc.sync.dma_start(
                        wk[:], wk_dram[:, pair * DC * P:(pair + 1) * DC * P])
                    wq = wp.tile([P, DC * P], bf16, tag="wx",
                                 name=f"wqt_{which}{pair}")
                    nc.sync.dma_start(
                        wq[:], wq_dram[:, pair * DC * P:(pair + 1) * DC * P])
                    st['wk'], st['wq'] = wk, wq
                    st['kt'] = ktp.tile([P, T], bf16, tag="kt",
                                        name=f"kt_{which}{pair}")
                    st['qt'] = qtp.tile([P, OWN], bf16, tag="qt",
                                        name=f"qt_{which}{pair}")
                def s_kchunk(c):
                    def f():
                        ps = linps.tile([P, 512], f32, tag="lin")
                        for d in range(DC):
                            nc.tensor.matmul(
                                ps[:], st['wk'][:, d * P:(d + 1) * P],
                                xT_cur[0][:, d * T + c * 512:
                                          d * T + (c + 1) * 512],
                                start=(d == 0), stop=(d == DC - 1))
                        nc.vector.tensor_copy(st['kt'][:, c * 512:
                                                       (c + 1) * 512], ps[:])
                    return f
                def s_qblock(si):
                    def f():
                        if si == 0:
                            st['qps'] = linps.tile([P, 512], f32, tag="lin",
                                                   name=f"qps_{which}{pair}")
                        for d in range(DC):
                            nc.tensor.matmul(
                                st['qps'][:, si * P:(si + 1) * P],
                                st['wq'][:, d * P:(d + 1) * P],
                                rhs_of(d, si),
                                start=(d == 0), stop=(d == DC - 1))
                        if si == NQB - 1:
                            nc.vector.tensor_copy(st['qt'][:], st['qps'][:])
                    return f
                steps = [s_dma] + [s_kchunk(c) for c in range(T // 512)] +                         [s_qblock(si) for si in range(NQB)]
                return steps, st

            def emit_attention(pair, qt, kt, vsb, causal, fill_steps):
                """One head pair's attention -> packed normalized bf16
                [P, OWN]. Software-pipelined: scores(kl+1) are emitted before
                AV(kl); `fill_steps` (next pair's K/Q projection groups) are
                drained one per kl iteration."""
                avtA = avps.tile([P, OWN], f32, tag="avt",
                                 name=f"avtA_{causal}{pair}")
                avtB = avps.tile([P, OWN], f32, tag="avt",
                                 name=f"avtB_{causal}{pair}")
                fill = list(fill_steps)
                pbs = {}

                def emit_scores(kl):
                    s0 = s0_of(kl) if causal else 0
                    c0 = s0 * P
                    sm = kl // NQB
                    sc = scps.tile([P, 2 * OWN], f32, tag="sc")
                    for hh in range(2):
                        hb = hh * HD
                        nc.tensor.matmul(
                            sc[:, hh * OWN + c0:(hh + 1) * OWN],
                            kt[hb:hb + HD, kl * P:(kl + 1) * P],
                            qt[hb:hb + HD, c0:OWN],
                            start=True, stop=(not causal),
                            skip_group_check=causal)
                        if causal:
                            nc.tensor.matmul(
                                sc[:, hh * OWN + sm * P:
                                   hh * OWN + (sm + 1) * P],
                                identb[:], maskt[:, kl * P:(kl + 1) * P],
                                start=False, stop=True,
                                skip_group_check=True)
                    pb = pbp.tile([P, 2 * OWN], bf16, tag="pb")
                    scv = sc[:].rearrange("p (h q) -> p h q", q=OWN)
                    pbv = pb[:].rearrange("p (h q) -> p h q", q=OWN)
                    nc.scalar.activation(pbv[:, :, c0:], scv[:, :, c0:],
                                         AF.Exp, scale=scale)
                    pbs[kl] = (pb, c0)

                def emit_av(kl, ki):
                    pb, c0 = pbs.pop(kl)
                    vbase = kl * PAIRS * 130 + pair * 130
                    nc.tensor.matmul(
                        avtA[0:65, c0:], vsb[:, vbase:vbase + 65],
                        pb[:, c0:OWN],
                        start=(ki == 0), stop=(ki == KB - 1),
                        skip_group_check=True)
                    nc.tensor.matmul(
                        avtB[0:65, c0:], vsb[:, vbase + 65:vbase + 130],
                        pb[:, OWN + c0:2 * OWN],
                        start=(ki == 0), stop=(ki == KB - 1),
                        skip_group_check=True)

                emit_scores(0)
                for ki in range(KB):
                    if ki + 1 < KB:
                        emit_scores(ki + 1)
                    emit_av(ki, ki)
                    if fill and (ki % 2 == 1 or ki == 2):
                        fill.pop(0)()
                while fill:
                    fill.pop(0)()

                # normalize by denominator row 64; head B first so its
                # partition-shift DMA overlaps head A's normalize chain
                recB = normp.tile([P, OWN], f32r, tag="rec", name="recB")
                nc.vector.reciprocal(recB[64:65, :], avtB[64:65, :])
                bcB = linps.tile([P, 512], f32, tag="lin")
                nc.tensor.matmul(bcB[0:HD, :], onesr[64:65, :],
                                 recB[64:65, :], start=True, stop=True)
                bcsB = normp.tile([P, OWN], bf16, tag="bcs", name="bcsB")
                nc.vector.tensor_copy(bcsB[0:HD, :], bcB[0:HD, :])
                avpk = avkp.tile([P, OWN], bf16, tag="avpk",
                                 name=f"avpk_{causal}{pair}")
                tmpb = tmpp.tile([P, OWN], bf16, tag="tmpb")
                nc.vector.tensor_mul(tmpb[0:HD, :], avtB[0:HD, :],
                                     bcsB[0:HD, :])
                # partition shift 0:64 -> 64:128 (only DMA can do this)
                nc.sync.dma_start(avpk[HD:P, :], tmpb[0:HD, :])
                recA = normp.tile([P, OWN], f32r, tag="rec", name="recA")
                nc.vector.reciprocal(recA[64:65, :], avtA[64:65, :])
                bcA = linps.tile([P, 512], f32, tag="lin")
                nc.tensor.matmul(bcA[0:HD, :], onesr[64:65, :],
                                 recA[64:65, :], start=True, stop=True)
                bcsA = normp.tile([P, OWN], bf16, tag="bcs", name="bcsA")
                nc.vector.tensor_copy(bcsA[0:HD, :], bcA[0:HD, :])
                nc.vector.tensor_mul(avpk[0:HD, :], avtA[0:HD, :],
                                     bcsA[0:HD, :])
                return avpk

            def attention_phase(wk_dram, wq_dram, rhs_of, xT, vsb, causal,
                                which):
                xT_cur[0] = xT
                steps, st = kq_steps(0, which, wk_dram, wq_dram, rhs_of)
                for step in steps:
                    step()
                avs = []
                for pair in range(PAIRS):
                    kt, qt = st['kt'], st['qt']
                    if pair + 1 < PAIRS:
                        nsteps, st = kq_steps(pair + 1, which, wk_dram,
                                              wq_dram, rhs_of)
                    else:
                        nsteps = []
                    avs.append(emit_attention(pair, qt, kt, vsb, causal,
                                              nsteps))
                return avs

            def emit_oproj_residual(wo_dram, bo_t, avpks, res_tiles, which):
                """res += transpose(Wo^T @ AV + bo); the transpose of chunk
                m-1 is emitted after chunk m's matmuls so the PE never waits
                on the ACT bias-evict."""
                pend = []

                def flush(m_out):
                    ev = pend.pop(0)
                    ps2 = linps.tile([P, 512], f32, tag="lin",
                                     name=f"ops2_{which}{m_out}")
                    ps2b = ps2[:].bitcast(bf16)
                    for si in range(NQB):
                        nc.tensor.transpose(ps2b[:, si * P:(si + 1) * P],
                                            ev[:, si * P:(si + 1) * P],
                                            identb[:])
                    for si in range(NQB):
                        nc.vector.tensor_add(
                            res_tiles[si][:, m_out * P:(m_out + 1) * P],
                            ps2b[:, si * P:(si + 1) * P],
                            res_tiles[si][:, m_out * P:(m_out + 1) * P])

                for m in range(DC):
                    wot = wop.tile([P, PAIRS * P], bf16, tag="wot",
                                   name=f"wot_{which}{m}")
                    nc.sync.dma_start(
                        wot[:], wo_dram[:, m * PAIRS * P:(m + 1) * PAIRS * P])
                    ps = linps.tile([P, 512], f32, tag="lin",
                                    name=f"ops_{which}{m}")
                    for pr in range(PAIRS):
                        nc.tensor.matmul(
                            ps[:], wot[:, pr * P:(pr + 1) * P], avpks[pr][:],
                            start=(pr == 0), stop=(pr == PAIRS - 1))
                    ev = evp.tile([P, OWN], bf16, tag="ev")
                    nc.scalar.activation(ev[:], ps[:], AF.Identity,
                                         bias=bo_t[:, m:m + 1])
                    pend.append(ev)
                    if m >= 1:
                        flush(m - 1)
                flush(DC - 1)

            def emit_lnq(res_tiles, gbkey):
                """LN own tokens + transpose -> [P, DC*OWN] bf16 mega-tile."""
                lnq = lnqp.tile([P, DC * OWN], bf16, tag="lnq")
                lnqv = lnq[:].rearrange("p (d t) -> p d t", t=OWN)
                for s in range(NQB):
                    lnt = emit_ln(res_tiles[s][:])
                    emit_transposes(lnt, lnqv[:, :, s * P:(s + 1) * P], gbkey)
                return lnq

            # ================= pipeline =================
            # ---- self-attention ----
            xT, vsb = stream_to_xT(x_dec, 1, wv_sa, "sa")
            res = []
            for si in range(NQB):
                t = residp.tile([P, D], f32, tag="resid")
                nc.sync.dma_start(t[:], x_own[si * P:(si + 1) * P, :])
                res.append(t)
            lnq1 = emit_lnq(res, 1)
            lnq1_rhs = lambda d, si: lnq1[:, d * OWN + si * P:
                                          d * OWN + (si + 1) * P]
            av_sa = attention_phase(wk_sa, wq_sa, lnq1_rhs, xT, vsb, True,
                                    "sa")
            emit_oproj_residual(wo_sa, bo_sa_t, av_sa, res, "sa")

            # ---- cross-attention (keys/values from RAW encoder_x) ----
            xTe, vsbe = stream_to_xT(x_enc, None, wv_ca, "ca")
            lnq2 = emit_lnq(res, 2)
            lnq2_rhs = lambda d, si: lnq2[:, d * OWN + si * P:
                                          d * OWN + (si + 1) * P]
            av_ca = attention_phase(wk_ca, wq_ca, lnq2_rhs, xTe, vsbe, False,
                                    "ca")
            emit_oproj_residual(wo_ca, bo_ca_t, av_ca, res, "ca")

            # ---- FFN ----
            lnq3 = emit_lnq(res, 3)
            rT = rtp.tile([P, FC * OWN], bf16, tag="rT")
            for f in range(FC):
                w1t = wp.tile([P, DC * P], bf16, tag="wx", name=f"w1t{f}")
                nc.sync.dma_start(
                    w1t[:], w1[:, f * DC * P:(f + 1) * DC * P])
                ps = linps.tile([P, 512], f32, tag="lin")
                for d in range(DC):
                    nc.tensor.matmul(
                        ps[:], w1t[:, d * P:(d + 1) * P],
                        lnq3[:, d * OWN:(d + 1) * OWN],
                        start=(d == 0), stop=(d == DC - 1))
                nc.scalar.activation(rT[:, f * OWN:(f + 1) * OWN], ps[:],
                                     AF.Relu, bias=b1_t[:, f:f + 1])
            pend2 = []

            def flush2(m_out):
                ev = pend2.pop(0)
                ps2 = linps.tile([P, 512], f32, tag="lin",
                                 name=f"fps2_{m_out}")
                ps2b = ps2[:].bitcast(bf16)
                for si in range(NQB):
                    nc.tensor.transpose(ps2b[:, si * P:(si + 1) * P],
                                        ev[:, si * P:(si + 1) * P],
                                        identb[:])
                for si in range(NQB):
                    nc.vector.tensor_add(res[si][:, m_out * P:(m_out + 1) * P],
                                         ps2b[:, si * P:(si + 1) * P],
                                         res[si][:, m_out * P:(m_out + 1) * P])

            for m in range(DC):
                w2t = w2p.tile([P, FC * P], bf16, tag="w2t", name=f"w2t{m}")
                nc.sync.dma_start(
                    w2t[:], w2[:, m * FC * P:(m + 1) * FC * P])
                ps = linps.tile([P, 512], f32, tag="lin", name=f"fps_{m}")
                for fi in range(FC):
                    nc.tensor.matmul(
                        ps[:], w2t[:, fi * P:(fi + 1) * P],
                        rT[:, fi * OWN:(fi + 1) * OWN],
                        start=(fi == 0), stop=(fi == FC - 1))
                ev = evp.tile([P, OWN], bf16, tag="ev")
                nc.scalar.activation(ev[:], ps[:], AF.Identity,
                                     bias=b2_t[:, m:m + 1])
                pend2.append(ev)
                if m >= 1:
                    flush2(m - 1)
            flush2(DC - 1)
            for si in range(NQB):
                nc.sync.dma_start(out[si * P:(si + 1) * P, :], res[si][:])

    nc.compile()
    return nc


def own_token_rows(cfg, j):
    return np.concatenate(
        [np.arange(P * (cfg.NQB * s + j), P * (cfg.NQB * s + j) + P)
         for s in range(cfg.NQB)])


def build_masks(cfg, j):
    """[P, KB*P] bf16: block kl = additive mask for own q-block s=kl//4 vs
    key block kl (only blocks kl <= 4s+3 are read by the program)."""
    m = np.zeros((cfg.KB, P, P), np.float32)
    for kl in range(cfg.KB):
        s = kl // cfg.NQB
        g = cfg.NQB * s + j                   # own block's global index
        if kl < g:
            continue
        elif kl == g:
            kidx = np.arange(P)[:, None]
            qidx = np.arange(P)[None, :]
            m[kl] = np.where(kidx <= qidx, 0.0, NEG)
        else:
            m[kl] = NEG
    out = m.transpose(1, 0, 2).reshape(P, -1)
    return np.ascontiguousarray(out).astype(ml_dtypes.bfloat16)


def _pack_pair_proj(w, cfg):
    """[H, D, HD] -> [128, PAIRS*DC*128] bf16."""
    w = np.asarray(w, np.float32)
    cat = np.stack([np.concatenate([w[2 * p], w[2 * p + 1]], axis=1)
                    for p in range(cfg.PAIRS)])          # [PR, D, 128]
    v = cat.reshape(cfg.PAIRS, cfg.DC, P, P)             # [PR, d, p, c]
    v = v.transpose(2, 0, 1, 3).reshape(P, -1)           # [p, PR*d*c]
    return np.ascontiguousarray(v).astype(ml_dtypes.bfloat16)


def _pack_v(w, cfg):
    """[H, D, HD] -> [128, VCH*DC*512] bf16."""
    w = np.asarray(w, np.float32)
    vall = w.transpose(1, 0, 2).reshape(cfg.D, cfg.H * HD)   # [D, H*64]
    v = vall.reshape(cfg.DC, P, cfg.VCH, 512)                # [d, p, nch, c]
    v = v.transpose(1, 2, 0, 3).reshape(P, -1)               # [p, nch*d*c]
    return np.ascontiguousarray(v).astype(ml_dtypes.bfloat16)


def _pack_o(w, cfg):
    """[D, D] -> [128, DC*PAIRS*128] bf16."""
    w = np.asarray(w, np.float32)
    v = w.reshape(cfg.PAIRS, P, cfg.DC, P)               # [pr, p, m, c]
    v = v.transpose(1, 2, 0, 3).reshape(P, -1)           # [p, m*pr*c]
    return np.ascontiguousarray(v).astype(ml_dtypes.bfloat16)


def _pack_w1(w, cfg):
    """[D, FF] -> [128, FC*DC*128]."""
    w = np.asarray(w, np.float32)
    v = w.reshape(cfg.DC, P, cfg.FC, P)                  # [d, p, f, c]
    v = v.transpose(1, 2, 0, 3).reshape(P, -1)           # [p, f*d*c]
    return np.ascontiguousarray(v).astype(ml_dtypes.bfloat16)


def _pack_w2(w, cfg):
    """[FF, D] -> [128, DC*FC*128]."""
    w = np.asarray(w, np.float32)
    v = w.reshape(cfg.FC, P, cfg.DC, P)                  # [fi, p, m, c]
    v = v.transpose(1, 2, 0, 3).reshape(P, -1)           # [p, m*fi*c]
    return np.ascontiguousarray(v).astype(ml_dtypes.bfloat16)


def prep_core_inputs(cfg, inputs, core):
    """Host-side slicing/packing for one core."""
    b, j = core // 4, core % 4
    a = lambda x: np.asarray(x)
    f32c = lambda x: np.ascontiguousarray(a(x), dtype=np.float32)
    rows = own_token_rows(cfg, j)
    return {
        "x_dec": f32c(a(inputs["decoder_x"])[b]),
        "x_enc": f32c(a(inputs["encoder_x"])[b]),
        "x_own": f32c(a(inputs["decoder_x"])[b][rows]),
        "wq_sa": _pack_pair_proj(inputs["Wq_sa"], cfg),
        "wk_sa": _pack_pair_proj(inputs["Wk_sa"], cfg),
        "wv_sa": _pack_v(inputs["Wv_sa"], cfg),
        "wo_sa": _pack_o(inputs["Wo_sa"], cfg),
        "bo_sa": f32c(inputs["bo_sa"]),
        "wq_ca": _pack_pair_proj(inputs["Wq_ca"], cfg),
        "wk_ca": _pack_pair_proj(inputs["Wk_ca"], cfg),
        "wv_ca": _pack_v(inputs["Wv_ca"], cfg),
        "wo_ca": _pack_o(inputs["Wo_ca"], cfg),
        "bo_ca": f32c(inputs["bo_ca"]),
        "w1": _pack_w1(inputs["W1"], cfg),
        "b1": f32c(inputs["b1"]),
        "w2": _pack_w2(inputs["W2"], cfg),
        "b2": f32c(inputs["b2"]),
        "masks": build_masks(cfg, j),
    }, rows


def gb_trivial(inputs):
    return all(np.allclose(np.asarray(inputs[g]), 1.0)
               for g in ("g1", "g2", "g3")) and \
           all(np.allclose(np.asarray(inputs[b]), 0.0)
               for b in ("be1", "be2", "be3"))


def run(inputs, trace=False, **rk):
    """Build + run on 8 cores; returns (full_output, BassKernelResults)."""
    from concourse.bass_utils import run_bass_kernel_spmd

    cfg = Cfg()
    with_gb = not gb_trivial(inputs)
    nc = build_nc(cfg, with_gb)

    in_maps, rows_all = [], []
    for core in range(8):
        im, rows = prep_core_inputs(cfg, inputs, core)
        if with_gb:
            for n in ("g1", "be1", "g2", "be2", "g3", "be3"):
                im[n] = np.ascontiguousarray(np.asarray(inputs[n]),
                                             dtype=np.float32)
        in_maps.append(im)
        rows_all.append(rows)

    res = run_bass_kernel_spmd(nc, in_maps, list(range(8)), trace=trace, **rk)
    full = np.zeros((2, cfg.T, cfg.D), np.float32)
    for core in range(8):
        full[core // 4][rows_all[core]] = res.results[core]["out"]
    return full, res


def kernel(**inputs) -> np.ndarray:
    return run(inputs)[0]
